# revision 12
# baseline (speedup 1.0000x reference)
"""Chamfer distance loss kernel for Trainium2 (8 NeuronCores).

Strategy
--------
reference: D[i,j] = ||pred_i - gt_j||^2 ; out = mean_i min_j D + mean_j min_i D.

We decompose into 8 independent jobs (4 batches x 2 directions), one per core.
For one job (query set A, candidate set B, both of size N=8192):

  * Host sorts A and B by x-coordinate.  For query rank i, the true nearest
    neighbor is almost always within a small rank window of i in the sorted
    B order.  Each 128-row query tile t scans the candidate window
    [128t - WL, 128t + SPAN - WL) (clamped via padding), SPAN wide.
  * The kernel computes, per query row, max_j (2<a,b_j> - ||b_j||^2) over the
    window via a K=4 TensorE matmul (features [2ax,2ay,2az,1] x [bx,by,bz,-||b||^2])
    and a VectorE free-axis max-reduce.  Then
    min_j D = ||a||^2 - rowmax, computed on host.
  * Exactness certificate (host): for query i with window [lo, hi), any
    excluded candidate j < lo has |a_x - b_x[j]| >= a_x - b_x[lo-1], so if
    band_min <= (x-margin)^2 on both sides the band min is the true min.
    The few rows that fail the certificate get an exact host-side scan.

Cores: core = 2*batch + direction (0: pred->gt, 1: gt->pred).
"""

import os

import numpy as np

import concourse.bass as bass
import concourse.tile as tile
from concourse import bacc, mybir
from concourse.bass_utils import run_bass_kernel_spmd

N = 8192  # points per cloud (both pred and gt)
B = 4  # batches
ROWT = 128  # query rows per tile
NTILES = N // ROWT  # 64
SPAN = 384  # candidate window width per row tile
WL = 128  # left extension of the window
WR = SPAN - WL - ROWT  # 192: right extension
PADDED = WL + N + WR  # padded candidate count
PAD_COORD = 1000.0  # sentinel coordinate for padding (never wins a min)

_CACHE = {}

# test.py introspection: set to BassKernelResults of the last run
LAST_RESULTS = None


def _build_program():
    nc = bacc.Bacc(
        "TRN2", target_bir_lowering=False, debug=False, num_devices=8
    )
    # float32r: same fp32 bits, PE streams it in single-pass 4-byte mode
    # (1 cycle/row for moving dim >= 256) vs float32's 2x half-speed passes.
    qfeat = nc.declare_dram_parameter(
        "qfeat", [4, N], mybir.dt.float32, isOutput=False
    )
    cfeat = nc.declare_dram_parameter(
        "cfeat", [4, PADDED], mybir.dt.float32, isOutput=False
    )
    rowmax_out = nc.declare_dram_parameter(
        "rowmax", [ROWT, NTILES], mybir.dt.float32, isOutput=True
    )

    with tile.TileContext(nc) as tc:
        with (
            tc.tile_pool(name="feats", bufs=1) as feats,
            tc.tile_pool(name="psum", bufs=2, space="PSUM") as psum_pool,
            tc.tile_pool(name="outp", bufs=1) as outp,
        ):
            # Operands replicated at partition bases 0/32/64/96 so four
            # matmuls can run concurrently in the four 32-row PE groups
            # (tile_position row packing; K=4 uses only 4 of 32 rows).
            # DMAs spread across engine DGE queues to run in parallel.
            q_sb = feats.tile([128, N], mybir.dt.float32, tag="q")
            c_sb = feats.tile([128, PADDED], mybir.dt.float32, tag="c")
            # Only SP (sync) and Activation (scalar) have HWDGE queues;
            # gpsimd SWDGE is ~1us/descriptor and far too slow.
            dma_engines = [nc.sync, nc.scalar]
            for j in range(4):
                dma_engines[j % 2].dma_start(
                    out=q_sb[32 * j : 32 * j + 4, :], in_=qfeat[:]
                )
                dma_engines[(j + 1) % 2].dma_start(
                    out=c_sb[32 * j : 32 * j + 4, :], in_=cfeat[:]
                )

            rmax = outp.tile([ROWT, NTILES], mybir.dt.float32)

            # 4 row-tiles share one 4-bank PSUM tensor; a single
            # TENSOR_REDUCE with a 3D AP [128, 4, SPAN] reduces all 4
            # (out free size 4), amortizing the per-op PSUM overhead.
            for g in range(NTILES // 4):
                ps = psum_pool.tile([ROWT, 4, 512], mybir.dt.float32)
                for j in range(4):
                    t = 4 * g + j
                    nc.tensor.matmul(
                        ps[:, j, :SPAN],
                        lhsT=q_sb[32 * j : 32 * j + 4, ROWT * t : ROWT * t + ROWT],
                        rhs=c_sb[32 * j : 32 * j + 4, ROWT * t : ROWT * t + SPAN],
                        start=True,
                        stop=True,
                        tile_position=(32 * j, 0),
                    )
                nc.vector.reduce_max(
                    rmax[:, 4 * g : 4 * g + 4],
                    ps[:, :, :SPAN],
                    axis=mybir.AxisListType.X,
                )

            nc.sync.dma_start(out=rowmax_out[:], in_=rmax[:])
    nc.compile()
    return nc


def _job_arrays(A, Bset):
    """Build sorted feature arrays for one (query=A, candidate=B) job."""
    ao = np.argsort(A[:, 0], kind="stable")
    bo = np.argsort(Bset[:, 0], kind="stable")
    As = np.ascontiguousarray(A[ao])
    Bs = np.ascontiguousarray(Bset[bo])

    qfeat = np.empty((4, N), np.float32)
    qfeat[0:3] = (2.0 * As).T
    qfeat[3] = 1.0

    cfeat = np.empty((4, PADDED), np.float32)
    cfeat[0:3] = PAD_COORD
    cfeat[3] = -3.0 * PAD_COORD * PAD_COORD
    cfeat[0:3, WL : WL + N] = Bs.T
    cfeat[3, WL : WL + N] = -(Bs.astype(np.float64) ** 2).sum(1).astype(np.float32)
    return As, Bs, qfeat, cfeat


def kernel(pred: np.ndarray, gt: np.ndarray) -> np.ndarray:
    global LAST_RESULTS
    pred = np.asarray(pred, dtype=np.float32)
    gt = np.asarray(gt, dtype=np.float32)
    assert pred.shape == (B, N, 3) and gt.shape == (B, N, 3)

    if "nc" not in _CACHE:
        _CACHE["nc"] = _build_program()
    nc = _CACHE["nc"]

    jobs = []
    in_maps = []
    for b in range(B):
        for A, Bset in ((pred[b], gt[b]), (gt[b], pred[b])):
            As, Bs, qfeat, cfeat = _job_arrays(A, Bset)
            jobs.append((As, Bs))
            in_maps.append({"qfeat": qfeat, "cfeat": cfeat})

    trace = bool(int(os.environ.get("CHAMFER_TRACE", "0")))
    bk = run_bass_kernel_spmd(nc, in_maps, list(range(8)), trace=trace)
    LAST_RESULTS = bk
    results = bk.results

    # Host: undo the rowmax formulation, certify, fix up, and average.
    total = 0.0
    i = np.arange(N)
    t = i // ROWT
    lo = ROWT * t - WL  # window start (unpadded coords, may be < 0)
    hi = ROWT * t + (SPAN - WL)  # window end (may be > N)
    for (As, Bs), r in zip(jobs, results):
        rowmax = np.asarray(r["rowmax"])  # [128, 64]
        asq = (As.astype(np.float64) ** 2).sum(1)
        d_band = asq - rowmax.T.reshape(-1).astype(np.float64)

        bx = Bs[:, 0].astype(np.float64)
        ax = As[:, 0].astype(np.float64)
        lmarg = np.where(lo >= 1, ax - bx[np.clip(lo - 1, 0, N - 1)], np.inf)
        rmarg = np.where(hi < N, bx[np.clip(hi, 0, N - 1)] - ax, np.inf)
        marg = np.minimum(lmarg, rmarg)
        ok = (marg >= 0) & (d_band <= marg * marg)
        bad = np.flatnonzero(~ok)
        if bad.size:
            Ad = As[bad].astype(np.float64)
            Bd = Bs.astype(np.float64)
            d = ((Ad[:, None, :] - Bd[None, :, :]) ** 2).sum(-1)
            d_band[bad] = d.min(1)
        total += d_band.mean()

    return np.float32(total / B)


# revision 19
# speedup vs baseline: 1.3224x; 1.3224x over previous
"""Chamfer distance loss kernel for Trainium2 (8 NeuronCores).

Strategy
--------
reference: D[i,j] = ||pred_i - gt_j||^2 ; out = mean_i min_j D + mean_j min_i D.

We decompose into 8 independent jobs (4 batches x 2 directions), one per core.
For one job (query set A, candidate set B, both of size N=8192):

  * Host sorts A and B by x-coordinate.  For query rank i, the true nearest
    neighbor is almost always within a small rank window of i in the sorted
    B order.  Each 128-row query tile t scans the candidate window
    [128t - WL, 128t + SPAN - WL) (clamped via padding), SPAN wide.
  * The kernel computes, per query row, max_j (2<a,b_j> - ||b_j||^2) over the
    window via a K=4 TensorE matmul (features [2ax,2ay,2az,1] x [bx,by,bz,-||b||^2])
    and a VectorE free-axis max-reduce.  Then
    min_j D = ||a||^2 - rowmax, computed on host.
  * Exactness certificate (host): for query i with window [lo, hi), any
    excluded candidate j < lo has |a_x - b_x[j]| >= a_x - b_x[lo-1], so if
    band_min <= (x-margin)^2 on both sides the band min is the true min.
    The few rows that fail the certificate get an exact host-side scan.

Cores: core = 2*batch + direction (0: pred->gt, 1: gt->pred).
"""

import os

import numpy as np

import concourse.bass as bass
import concourse.tile as tile
from concourse import bacc, mybir
from concourse.bass_utils import run_bass_kernel_spmd

N = 8192  # points per cloud (both pred and gt)
B = 4  # batches
ROWT = 128  # query rows per tile
NTILES = N // ROWT  # 64
SPAN = 384  # candidate window width per row tile
WL = 128  # left extension of the window
WR = SPAN - WL - ROWT  # 192: right extension
PADDED = WL + N + WR  # padded candidate count
PAD_COORD = 1000.0  # sentinel coordinate for padding (never wins a min)

_CACHE = {}

# test.py introspection: set to BassKernelResults of the last run
LAST_RESULTS = None


NGROUP = NTILES // 4  # 16 row-tiles per PE row group


def _build_program():
    nc = bacc.Bacc(
        "TRN2", target_bir_lowering=False, debug=False, num_devices=8
    )
    # Row group j (PE rows 32j..32j+4, via tile_position) handles row-tiles
    # t = 4g + j.  Both matmul operands must live at partitions 32j..32j+4,
    # so the host ships per-group gathered slices: group j's query columns
    # [4, 16*128] and its candidate windows [4, 16*SPAN] — no duplicated
    # bytes.  Four matmuls (one per group) run concurrently on the PE.
    qfeats = [
        nc.declare_dram_parameter(
            f"qfeat{j}", [4, NGROUP * ROWT], mybir.dt.float32, isOutput=False
        )
        for j in range(4)
    ]
    cfeats = [
        nc.declare_dram_parameter(
            f"cfeat{j}", [4, NGROUP * SPAN], mybir.dt.float32, isOutput=False
        )
        for j in range(4)
    ]
    rowmax_out = nc.declare_dram_parameter(
        "rowmax", [ROWT, NTILES], mybir.dt.float32, isOutput=True
    )

    with tile.TileContext(nc) as tc:
        with (
            tc.tile_pool(name="feats", bufs=1) as feats,
            tc.tile_pool(name="psum", bufs=2, space="PSUM") as psum_pool,
            tc.tile_pool(name="outp", bufs=1) as outp,
        ):
            # Only SP (sync) and Activation (scalar) have HWDGE queues;
            # gpsimd SWDGE is ~1us/descriptor and far too slow.
            dma_engines = [nc.sync, nc.scalar]
            q_sb = feats.tile([128, NGROUP * ROWT], mybir.dt.float32, tag="q")
            c_sb = feats.tile([128, NGROUP * SPAN], mybir.dt.float32, tag="c")
            for j in range(4):
                dma_engines[j % 2].dma_start(
                    out=q_sb[32 * j : 32 * j + 4, :], in_=qfeats[j][:]
                )
                dma_engines[(j + 1) % 2].dma_start(
                    out=c_sb[32 * j : 32 * j + 4, :], in_=cfeats[j][:]
                )

            rmax = outp.tile([ROWT, NTILES], mybir.dt.float32)

            # 4 row-tiles share one 4-bank PSUM tensor; a single
            # TENSOR_REDUCE with a 3D AP [128, 4, SPAN] reduces all 4
            # (out free size 4), amortizing the per-op PSUM overhead.
            for g in range(NGROUP):
                ps = psum_pool.tile([ROWT, 4, 512], mybir.dt.float32)
                for j in range(4):
                    nc.tensor.matmul(
                        ps[:, j, :SPAN],
                        lhsT=q_sb[
                            32 * j : 32 * j + 4, ROWT * g : ROWT * g + ROWT
                        ],
                        rhs=c_sb[32 * j : 32 * j + 4, SPAN * g : SPAN * g + SPAN],
                        start=True,
                        stop=True,
                        tile_position=(32 * j, 0),
                    )
                nc.vector.reduce_max(
                    rmax[:, 4 * g : 4 * g + 4],
                    ps[:, :, :SPAN],
                    axis=mybir.AxisListType.X,
                )

            nc.sync.dma_start(out=rowmax_out[:], in_=rmax[:])
    nc.compile()
    return nc


def _job_arrays(A, Bset):
    """Build per-row-group gathered feature arrays for one job."""
    ao = np.argsort(A[:, 0], kind="stable")
    bo = np.argsort(Bset[:, 0], kind="stable")
    As = np.ascontiguousarray(A[ao])
    Bs = np.ascontiguousarray(Bset[bo])

    qfeat = np.empty((4, N), np.float32)
    qfeat[0:3] = (2.0 * As).T
    qfeat[3] = 1.0

    cfeat = np.empty((4, PADDED), np.float32)
    cfeat[0:3] = PAD_COORD
    cfeat[3] = -3.0 * PAD_COORD * PAD_COORD
    cfeat[0:3, WL : WL + N] = Bs.T
    cfeat[3, WL : WL + N] = -(Bs.astype(np.float64) ** 2).sum(1).astype(np.float32)

    in_map = {}
    g = np.arange(NGROUP)
    for j in range(4):
        t = 4 * g + j
        qidx = (ROWT * t)[:, None] + np.arange(ROWT)[None, :]
        in_map[f"qfeat{j}"] = np.ascontiguousarray(
            qfeat[:, qidx].reshape(4, NGROUP * ROWT)
        )
        cidx = (ROWT * t)[:, None] + np.arange(SPAN)[None, :]
        in_map[f"cfeat{j}"] = np.ascontiguousarray(
            cfeat[:, cidx].reshape(4, NGROUP * SPAN)
        )
    return As, Bs, in_map


def kernel(pred: np.ndarray, gt: np.ndarray) -> np.ndarray:
    global LAST_RESULTS
    pred = np.asarray(pred, dtype=np.float32)
    gt = np.asarray(gt, dtype=np.float32)
    assert pred.shape == (B, N, 3) and gt.shape == (B, N, 3)

    if "nc" not in _CACHE:
        _CACHE["nc"] = _build_program()
    nc = _CACHE["nc"]

    jobs = []
    in_maps = []
    for b in range(B):
        for A, Bset in ((pred[b], gt[b]), (gt[b], pred[b])):
            As, Bs, in_map = _job_arrays(A, Bset)
            jobs.append((As, Bs))
            in_maps.append(in_map)

    trace = bool(int(os.environ.get("CHAMFER_TRACE", "0")))
    bk = run_bass_kernel_spmd(nc, in_maps, list(range(8)), trace=trace)
    LAST_RESULTS = bk
    results = bk.results

    # Host: undo the rowmax formulation, certify, fix up, and average.
    total = 0.0
    i = np.arange(N)
    t = i // ROWT
    lo = ROWT * t - WL  # window start (unpadded coords, may be < 0)
    hi = ROWT * t + (SPAN - WL)  # window end (may be > N)
    for (As, Bs), r in zip(jobs, results):
        rowmax = np.asarray(r["rowmax"])  # [128, 64]
        asq = (As.astype(np.float64) ** 2).sum(1)
        d_band = asq - rowmax.T.reshape(-1).astype(np.float64)

        bx = Bs[:, 0].astype(np.float64)
        ax = As[:, 0].astype(np.float64)
        lmarg = np.where(lo >= 1, ax - bx[np.clip(lo - 1, 0, N - 1)], np.inf)
        rmarg = np.where(hi < N, bx[np.clip(hi, 0, N - 1)] - ax, np.inf)
        marg = np.minimum(lmarg, rmarg)
        ok = (marg >= 0) & (d_band <= marg * marg)
        bad = np.flatnonzero(~ok)
        if bad.size:
            Ad = As[bad].astype(np.float64)
            Bd = Bs.astype(np.float64)
            d = ((Ad[:, None, :] - Bd[None, :, :]) ** 2).sum(-1)
            d_band[bad] = d.min(1)
        total += d_band.mean()

    return np.float32(total / B)


# revision 23
# speedup vs baseline: 1.7037x; 1.2884x over previous
"""Chamfer distance loss kernel for Trainium2 (8 NeuronCores).

Strategy
--------
reference: D[i,j] = ||pred_i - gt_j||^2 ; out = mean_i min_j D + mean_j min_i D.

We decompose into 8 independent jobs (4 batches x 2 directions), one per core.
For one job (query set A, candidate set B, both of size N=8192):

  * Host sorts A and B by x-coordinate.  For query rank i, the true nearest
    neighbor is almost always within a small rank window of i in the sorted
    B order.  Each 128-row query tile t scans the candidate window
    [128t - WL, 128t + SPAN - WL) (clamped via padding), SPAN wide.
  * The kernel computes, per query row, max_j (2<a,b_j> - ||b_j||^2) over the
    window via a K=4 TensorE matmul (features [2ax,2ay,2az,1] x [bx,by,bz,-||b||^2])
    and a VectorE free-axis max-reduce.  Then
    min_j D = ||a||^2 - rowmax, computed on host.
  * Exactness certificate (host): for query i with window [lo, hi), any
    excluded candidate j < lo has |a_x - b_x[j]| >= a_x - b_x[lo-1], so if
    band_min <= (x-margin)^2 on both sides the band min is the true min.
    The few rows that fail the certificate get an exact host-side scan.

Cores: core = 2*batch + direction (0: pred->gt, 1: gt->pred).
"""

import os

import numpy as np

import concourse.bass as bass
import concourse.tile as tile
from concourse import bacc, mybir
from concourse.bass_utils import run_bass_kernel_spmd

N = 8192  # points per cloud (both pred and gt)
B = 4  # batches
ROWT = 128  # query rows per tile
NTILES = N // ROWT  # 64
SPAN = 384  # candidate window width per row tile
WL = 128  # left extension of the window
WR = SPAN - WL - ROWT  # 192: right extension
PADDED = WL + N + WR  # padded candidate count
PAD_COORD = 1000.0  # sentinel coordinate for padding (never wins a min)

_CACHE = {}

# test.py introspection: set to BassKernelResults of the last run
LAST_RESULTS = None


NGROUP = NTILES // 4  # 16 row-tiles per PE row group


def _build_program():
    nc = bacc.Bacc(
        "TRN2", target_bir_lowering=False, debug=False, num_devices=8
    )
    # Row group j (PE rows 32j..32j+4, via tile_position) handles row-tiles
    # t = 4g + j.  Both matmul operands must live at partitions 32j..32j+4,
    # so the host ships per-group gathered slices: group j's query columns
    # [4, 16*128] and its candidate windows [4, 16*SPAN] — no duplicated
    # bytes.  Four matmuls (one per group) run concurrently on the PE.
    # Matmul operands must start at partition 32j (PE row-group base), so we
    # use K=32 matmuls over the full 32-partition strip of row group j:
    #   - c_sb (moving) is interleaved: partition 32j + 4m + f = feature f,
    #     row group j, column-chunk m (chunk m covers reduce-groups 2m,2m+1).
    #     Its DMA writes all 128 partitions -> full 16-port bandwidth.
    #   - q_stat (stationary) is zero-padded per reduce-group: the [32, 128]
    #     weight slice for (g, j) has query features only in rows 4*(g//2)..+4
    #     and zeros elsewhere, so the other 7 chunks in the moving strip are
    #     multiplied away exactly.
    CCH = 2 * SPAN  # c columns per chunk
    qfeat_d = nc.declare_dram_parameter(
        "qfeat", [128, NGROUP * ROWT], mybir.dt.float32, isOutput=False
    )
    cfeat_d = nc.declare_dram_parameter(
        "cfeat", [128, CCH], mybir.dt.float32, isOutput=False
    )
    rowmax_out = nc.declare_dram_parameter(
        "rowmax", [ROWT, NTILES], mybir.dt.float32, isOutput=True
    )

    with tile.TileContext(nc) as tc:
        with (
            tc.tile_pool(name="feats", bufs=1) as feats,
            tc.tile_pool(name="psum", bufs=2, space="PSUM") as psum_pool,
            tc.tile_pool(name="outp", bufs=1) as outp,
        ):
            q_sb = feats.tile([128, NGROUP * ROWT], mybir.dt.float32, tag="q")
            c_sb = feats.tile([128, CCH], mybir.dt.float32, tag="c")
            # Split each input across both HWDGE queues (sync + scalar).
            half_q = NGROUP * ROWT // 2
            half_c = CCH // 2
            nc.sync.dma_start(out=c_sb[:, :half_c], in_=cfeat_d[:, :half_c])
            nc.scalar.dma_start(out=c_sb[:, half_c:], in_=cfeat_d[:, half_c:])
            nc.scalar.dma_start(out=q_sb[:, :half_q], in_=qfeat_d[:, :half_q])
            nc.sync.dma_start(out=q_sb[:, half_q:], in_=qfeat_d[:, half_q:])

            rmax = outp.tile([ROWT, NTILES], mybir.dt.float32)

            # 4 row-tiles share one 4-bank PSUM tensor; a single
            # TENSOR_REDUCE with a 3D AP [128, 4, SPAN] reduces all 4
            # (out free size 4), amortizing the per-op PSUM overhead.
            for g in range(NGROUP):
                r = g % 2
                ps = psum_pool.tile([ROWT, 4, 512], mybir.dt.float32)
                for j in range(4):
                    p0 = 32 * j
                    nc.tensor.matmul(
                        ps[:, j, :SPAN],
                        lhsT=q_sb[p0 : p0 + 32, ROWT * g : ROWT * g + ROWT],
                        rhs=c_sb[p0 : p0 + 32, SPAN * r : SPAN * r + SPAN],
                        start=True,
                        stop=True,
                        tile_position=(32 * j, 0),
                    )
                nc.vector.reduce_max(
                    rmax[:, 4 * g : 4 * g + 4],
                    ps[:, :, :SPAN],
                    axis=mybir.AxisListType.X,
                )

            nc.sync.dma_start(out=rowmax_out[:], in_=rmax[:])
    nc.compile()
    return nc


def _job_arrays(A, Bset):
    """Build per-row-group gathered feature arrays for one job."""
    ao = np.argsort(A[:, 0], kind="stable")
    bo = np.argsort(Bset[:, 0], kind="stable")
    As = np.ascontiguousarray(A[ao])
    Bs = np.ascontiguousarray(Bset[bo])

    qfeat = np.empty((4, N), np.float32)
    qfeat[0:3] = (2.0 * As).T
    qfeat[3] = 1.0

    cfeat = np.empty((4, PADDED), np.float32)
    cfeat[0:3] = PAD_COORD
    cfeat[3] = -3.0 * PAD_COORD * PAD_COORD
    cfeat[0:3, WL : WL + N] = Bs.T
    cfeat[3, WL : WL + N] = -(Bs.astype(np.float64) ** 2).sum(1).astype(np.float32)

    # c_big interleaved: partition 32j + 4m + f = (feature f, row group j,
    # chunk m), chunk m covering reduce-groups {2m, 2m+1}.
    # q_stat zero-padded stationary: for reduce-group g, row group j, the
    # [32, 128] slice at columns 128g has features only in rows 4*(g//2)..+4.
    q_stat = np.zeros((128, NGROUP * ROWT), np.float32)
    c_big = np.empty((128, 2 * SPAN), np.float32)
    g = np.arange(NGROUP)
    for j in range(4):
        t = 4 * g + j
        cidx = (ROWT * t)[:, None] + np.arange(SPAN)[None, :]
        cj = cfeat[:, cidx]  # [4f, 16g, SPAN]
        c_big[32 * j : 32 * j + 32] = (
            cj.reshape(4, 8, 2 * SPAN).transpose(1, 0, 2).reshape(32, 2 * SPAN)
        )
        for gg in range(NGROUP):
            tt = 4 * gg + j
            m = gg // 2
            q_stat[
                32 * j + 4 * m : 32 * j + 4 * m + 4,
                ROWT * gg : ROWT * gg + ROWT,
            ] = qfeat[:, ROWT * tt : ROWT * tt + ROWT]
    in_map = {"qfeat": q_stat, "cfeat": c_big}
    return As, Bs, in_map


def kernel(pred: np.ndarray, gt: np.ndarray) -> np.ndarray:
    global LAST_RESULTS
    pred = np.asarray(pred, dtype=np.float32)
    gt = np.asarray(gt, dtype=np.float32)
    assert pred.shape == (B, N, 3) and gt.shape == (B, N, 3)

    if "nc" not in _CACHE:
        _CACHE["nc"] = _build_program()
    nc = _CACHE["nc"]

    jobs = []
    in_maps = []
    for b in range(B):
        for A, Bset in ((pred[b], gt[b]), (gt[b], pred[b])):
            As, Bs, in_map = _job_arrays(A, Bset)
            jobs.append((As, Bs))
            in_maps.append(in_map)

    trace = bool(int(os.environ.get("CHAMFER_TRACE", "0")))
    bk = run_bass_kernel_spmd(nc, in_maps, list(range(8)), trace=trace)
    LAST_RESULTS = bk
    results = bk.results

    # Host: undo the rowmax formulation, certify, fix up, and average.
    total = 0.0
    i = np.arange(N)
    t = i // ROWT
    lo = ROWT * t - WL  # window start (unpadded coords, may be < 0)
    hi = ROWT * t + (SPAN - WL)  # window end (may be > N)
    for (As, Bs), r in zip(jobs, results):
        rowmax = np.asarray(r["rowmax"])  # [128, 64]
        asq = (As.astype(np.float64) ** 2).sum(1)
        d_band = asq - rowmax.T.reshape(-1).astype(np.float64)

        bx = Bs[:, 0].astype(np.float64)
        ax = As[:, 0].astype(np.float64)
        lmarg = np.where(lo >= 1, ax - bx[np.clip(lo - 1, 0, N - 1)], np.inf)
        rmarg = np.where(hi < N, bx[np.clip(hi, 0, N - 1)] - ax, np.inf)
        marg = np.minimum(lmarg, rmarg)
        ok = (marg >= 0) & (d_band <= marg * marg)
        bad = np.flatnonzero(~ok)
        if bad.size:
            Ad = As[bad].astype(np.float64)
            Bd = Bs.astype(np.float64)
            d = ((Ad[:, None, :] - Bd[None, :, :]) ** 2).sum(-1)
            d_band[bad] = d.min(1)
        total += d_band.mean()

    return np.float32(total / B)


# revision 25
# speedup vs baseline: 1.9235x; 1.1290x over previous
"""Chamfer distance loss kernel for Trainium2 (8 NeuronCores).

Strategy
--------
reference: D[i,j] = ||pred_i - gt_j||^2 ; out = mean_i min_j D + mean_j min_i D.

We decompose into 8 independent jobs (4 batches x 2 directions), one per core.
For one job (query set A, candidate set B, both of size N=8192):

  * Host sorts A and B by x-coordinate.  For query rank i, the true nearest
    neighbor is almost always within a small rank window of i in the sorted
    B order.  Each 128-row query tile t scans the candidate window
    [128t - WL, 128t + SPAN - WL) (clamped via padding), SPAN wide.
  * The kernel computes, per query row, max_j (2<a,b_j> - ||b_j||^2) over the
    window via a K=4 TensorE matmul (features [2ax,2ay,2az,1] x [bx,by,bz,-||b||^2])
    and a VectorE free-axis max-reduce.  Then
    min_j D = ||a||^2 - rowmax, computed on host.
  * Exactness certificate (host): for query i with window [lo, hi), any
    excluded candidate j < lo has |a_x - b_x[j]| >= a_x - b_x[lo-1], so if
    band_min <= (x-margin)^2 on both sides the band min is the true min.
    The few rows that fail the certificate get an exact host-side scan.

Cores: core = 2*batch + direction (0: pred->gt, 1: gt->pred).
"""

import os

import numpy as np

import concourse.bass as bass
import concourse.tile as tile
from concourse import bacc, mybir
from concourse.bass_utils import run_bass_kernel_spmd

N = 8192  # points per cloud (both pred and gt)
B = 4  # batches
ROWT = 128  # query rows per tile
NTILES = N // ROWT  # 64
SPAN = 320  # candidate window width per row tile
WL = 96  # left extension of the window
WR = SPAN - WL - ROWT  # 192: right extension
PADDED = WL + N + WR  # padded candidate count
PAD_COORD = 1000.0  # sentinel coordinate for padding (never wins a min)

_CACHE = {}

# test.py introspection: set to BassKernelResults of the last run
LAST_RESULTS = None


NGROUP = NTILES // 4  # 16 row-tiles per PE row group


def _build_program():
    nc = bacc.Bacc(
        "TRN2", target_bir_lowering=False, debug=False, num_devices=8
    )
    # Row group j (PE rows 32j..32j+4, via tile_position) handles row-tiles
    # t = 4g + j.  Both matmul operands must live at partitions 32j..32j+4,
    # so the host ships per-group gathered slices: group j's query columns
    # [4, 16*128] and its candidate windows [4, 16*SPAN] — no duplicated
    # bytes.  Four matmuls (one per group) run concurrently on the PE.
    # Matmul operands must start at partition 32j (PE row-group base), so we
    # use K=32 matmuls over the full 32-partition strip of row group j:
    #   - c_sb (moving) is interleaved: partition 32j + 4m + f = feature f,
    #     row group j, column-chunk m (chunk m covers reduce-groups 2m,2m+1).
    #     Its DMA writes all 128 partitions -> full 16-port bandwidth.
    #   - q_stat (stationary) is zero-padded per reduce-group: the [32, 128]
    #     weight slice for (g, j) has query features only in rows 4*(g//2)..+4
    #     and zeros elsewhere, so the other 7 chunks in the moving strip are
    #     multiplied away exactly.
    CCH = 2 * SPAN  # c columns per chunk
    qfeat_d = nc.declare_dram_parameter(
        "qfeat", [128, NGROUP * ROWT], mybir.dt.float32, isOutput=False
    )
    cfeat_d = nc.declare_dram_parameter(
        "cfeat", [128, CCH], mybir.dt.float32, isOutput=False
    )
    rowmax_out = nc.declare_dram_parameter(
        "rowmax", [ROWT, NTILES], mybir.dt.float32, isOutput=True
    )

    with tile.TileContext(nc) as tc:
        with (
            tc.tile_pool(name="feats", bufs=1) as feats,
            tc.tile_pool(name="psum", bufs=2, space="PSUM") as psum_pool,
            tc.tile_pool(name="outp", bufs=1) as outp,
        ):
            # Separate tiles per input chunk so Tile's dependency tracking
            # lets early matmuls start while later chunks are still in
            # flight.  c splits by parity column (r), q by g-quarters.
            c_sbs = [
                feats.tile([128, SPAN], mybir.dt.float32, tag=f"c{r}", name=f"c{r}")
                for r in range(2)
            ]
            QQ = 4 * ROWT
            q_sbs = [
                feats.tile([128, QQ], mybir.dt.float32, tag=f"q{i}", name=f"q{i}")
                for i in range(4)
            ]
            nc.sync.dma_start(out=c_sbs[0][:], in_=cfeat_d[:, :SPAN])
            nc.scalar.dma_start(out=q_sbs[0][:], in_=qfeat_d[:, :QQ])
            nc.sync.dma_start(out=q_sbs[1][:], in_=qfeat_d[:, QQ : 2 * QQ])
            nc.scalar.dma_start(out=c_sbs[1][:], in_=cfeat_d[:, SPAN:])
            nc.sync.dma_start(out=q_sbs[2][:], in_=qfeat_d[:, 2 * QQ : 3 * QQ])
            nc.scalar.dma_start(out=q_sbs[3][:], in_=qfeat_d[:, 3 * QQ :])

            rmax = outp.tile([ROWT, NTILES], mybir.dt.float32)

            # 4 row-tiles share one 4-bank PSUM tensor; a single
            # TENSOR_REDUCE with a 3D AP [128, 4, SPAN] reduces all 4
            # (out free size 4), amortizing the per-op PSUM overhead.
            # Even reduce-groups first: they only need c chunk r=0.
            g_order = list(range(0, NGROUP, 2)) + list(range(1, NGROUP, 2))
            for g in g_order:
                r = g % 2
                q_sb = q_sbs[g // 4]
                qcol = ROWT * (g % 4)
                ps = psum_pool.tile([ROWT, 4, 512], mybir.dt.float32, tag="ps", name=f"ps{g}")
                for j in range(4):
                    p0 = 32 * j
                    nc.tensor.matmul(
                        ps[:, j, :SPAN],
                        lhsT=q_sb[p0 : p0 + 32, qcol : qcol + ROWT],
                        rhs=c_sbs[r][p0 : p0 + 32, :],
                        start=True,
                        stop=True,
                        tile_position=(32 * j, 0),
                    )
                nc.vector.reduce_max(
                    rmax[:, 4 * g : 4 * g + 4],
                    ps[:, :, :SPAN],
                    axis=mybir.AxisListType.X,
                )

            nc.sync.dma_start(out=rowmax_out[:], in_=rmax[:])
    nc.compile()
    return nc


def _job_arrays(A, Bset):
    """Build per-row-group gathered feature arrays for one job."""
    ao = np.argsort(A[:, 0], kind="stable")
    bo = np.argsort(Bset[:, 0], kind="stable")
    As = np.ascontiguousarray(A[ao])
    Bs = np.ascontiguousarray(Bset[bo])

    qfeat = np.empty((4, N), np.float32)
    qfeat[0:3] = (2.0 * As).T
    qfeat[3] = 1.0

    cfeat = np.empty((4, PADDED), np.float32)
    cfeat[0:3] = PAD_COORD
    cfeat[3] = -3.0 * PAD_COORD * PAD_COORD
    cfeat[0:3, WL : WL + N] = Bs.T
    cfeat[3, WL : WL + N] = -(Bs.astype(np.float64) ** 2).sum(1).astype(np.float32)

    # c_big interleaved: partition 32j + 4m + f = (feature f, row group j,
    # chunk m), chunk m covering reduce-groups {2m, 2m+1}.
    # q_stat zero-padded stationary: for reduce-group g, row group j, the
    # [32, 128] slice at columns 128g has features only in rows 4*(g//2)..+4.
    q_stat = np.zeros((128, NGROUP * ROWT), np.float32)
    c_big = np.empty((128, 2 * SPAN), np.float32)
    g = np.arange(NGROUP)
    for j in range(4):
        t = 4 * g + j
        cidx = (ROWT * t)[:, None] + np.arange(SPAN)[None, :]
        cj = cfeat[:, cidx]  # [4f, 16g, SPAN]
        c_big[32 * j : 32 * j + 32] = (
            cj.reshape(4, 8, 2 * SPAN).transpose(1, 0, 2).reshape(32, 2 * SPAN)
        )
        for gg in range(NGROUP):
            tt = 4 * gg + j
            m = gg // 2
            q_stat[
                32 * j + 4 * m : 32 * j + 4 * m + 4,
                ROWT * gg : ROWT * gg + ROWT,
            ] = qfeat[:, ROWT * tt : ROWT * tt + ROWT]
    in_map = {"qfeat": q_stat, "cfeat": c_big}
    return As, Bs, in_map


def kernel(pred: np.ndarray, gt: np.ndarray) -> np.ndarray:
    global LAST_RESULTS
    pred = np.asarray(pred, dtype=np.float32)
    gt = np.asarray(gt, dtype=np.float32)
    assert pred.shape == (B, N, 3) and gt.shape == (B, N, 3)

    if "nc" not in _CACHE:
        _CACHE["nc"] = _build_program()
    nc = _CACHE["nc"]

    jobs = []
    in_maps = []
    for b in range(B):
        for A, Bset in ((pred[b], gt[b]), (gt[b], pred[b])):
            As, Bs, in_map = _job_arrays(A, Bset)
            jobs.append((As, Bs))
            in_maps.append(in_map)

    trace = bool(int(os.environ.get("CHAMFER_TRACE", "0")))
    bk = run_bass_kernel_spmd(nc, in_maps, list(range(8)), trace=trace)
    LAST_RESULTS = bk
    results = bk.results

    # Host: undo the rowmax formulation, certify, fix up, and average.
    total = 0.0
    i = np.arange(N)
    t = i // ROWT
    lo = ROWT * t - WL  # window start (unpadded coords, may be < 0)
    hi = ROWT * t + (SPAN - WL)  # window end (may be > N)
    for (As, Bs), r in zip(jobs, results):
        rowmax = np.asarray(r["rowmax"])  # [128, 64]
        asq = (As.astype(np.float64) ** 2).sum(1)
        d_band = asq - rowmax.T.reshape(-1).astype(np.float64)

        bx = Bs[:, 0].astype(np.float64)
        ax = As[:, 0].astype(np.float64)
        lmarg = np.where(lo >= 1, ax - bx[np.clip(lo - 1, 0, N - 1)], np.inf)
        rmarg = np.where(hi < N, bx[np.clip(hi, 0, N - 1)] - ax, np.inf)
        marg = np.minimum(lmarg, rmarg)
        ok = (marg >= 0) & (d_band <= marg * marg)
        bad = np.flatnonzero(~ok)
        if bad.size:
            Ad = As[bad].astype(np.float64)
            Bd = Bs.astype(np.float64)
            d = ((Ad[:, None, :] - Bd[None, :, :]) ** 2).sum(-1)
            d_band[bad] = d.min(1)
        total += d_band.mean()

    return np.float32(total / B)


# revision 26
# speedup vs baseline: 2.1378x; 1.1114x over previous
"""Chamfer distance loss kernel for Trainium2 (8 NeuronCores).

Strategy
--------
reference: D[i,j] = ||pred_i - gt_j||^2 ; out = mean_i min_j D + mean_j min_i D.

We decompose into 8 independent jobs (4 batches x 2 directions), one per core.
For one job (query set A, candidate set B, both of size N=8192):

  * Host sorts A and B by x-coordinate.  For query rank i, the true nearest
    neighbor is almost always within a small rank window of i in the sorted
    B order.  Each 128-row query tile t scans the candidate window
    [128t - WL, 128t + SPAN - WL) (clamped via padding), SPAN wide.
  * The kernel computes, per query row, max_j (2<a,b_j> - ||b_j||^2) over the
    window via a K=4 TensorE matmul (features [2ax,2ay,2az,1] x [bx,by,bz,-||b||^2])
    and a VectorE free-axis max-reduce.  Then
    min_j D = ||a||^2 - rowmax, computed on host.
  * Exactness certificate (host): for query i with window [lo, hi), any
    excluded candidate j < lo has |a_x - b_x[j]| >= a_x - b_x[lo-1], so if
    band_min <= (x-margin)^2 on both sides the band min is the true min.
    The few rows that fail the certificate get an exact host-side scan.

Cores: core = 2*batch + direction (0: pred->gt, 1: gt->pred).
"""

import os

import numpy as np

import concourse.bass as bass
import concourse.tile as tile
from concourse import bacc, mybir
from concourse.bass_utils import run_bass_kernel_spmd

N = 8192  # points per cloud (both pred and gt)
B = 4  # batches
ROWT = 128  # query rows per tile
NTILES = N // ROWT  # 64
SPAN = 320  # candidate window width per row tile
WL = 96  # left extension of the window
WR = SPAN - WL - ROWT  # 192: right extension
PADDED = WL + N + WR  # padded candidate count
PAD_COORD = 1000.0  # sentinel coordinate for padding (never wins a min)

_CACHE = {}

# test.py introspection: set to BassKernelResults of the last run
LAST_RESULTS = None


NGROUP = NTILES // 4  # 16 row-tiles per PE row group


def _build_program():
    nc = bacc.Bacc(
        "TRN2", target_bir_lowering=False, debug=False, num_devices=8
    )
    # Row group j (PE rows 32j..32j+4, via tile_position) handles row-tiles
    # t = 4g + j.  Both matmul operands must live at partitions 32j..32j+4,
    # so the host ships per-group gathered slices: group j's query columns
    # [4, 16*128] and its candidate windows [4, 16*SPAN] — no duplicated
    # bytes.  Four matmuls (one per group) run concurrently on the PE.
    # Matmul operands must start at partition 32j (PE row-group base), so we
    # use K=32 matmuls over the full 32-partition strip of row group j:
    #   - c_sb (moving) is interleaved: partition 32j + 4m + f = feature f,
    #     row group j, column-chunk m (chunk m covers reduce-groups 2m,2m+1).
    #     Its DMA writes all 128 partitions -> full 16-port bandwidth.
    #   - q_stat (stationary) is zero-padded per reduce-group: the [32, 128]
    #     weight slice for (g, j) has query features only in rows 4*(g//2)..+4
    #     and zeros elsewhere, so the other 7 chunks in the moving strip are
    #     multiplied away exactly.
    CCH = 2 * SPAN  # c columns per chunk
    qfeat_d = nc.declare_dram_parameter(
        "qfeat", [128, NGROUP * ROWT], mybir.dt.float32r, isOutput=False
    )
    cfeat_d = nc.declare_dram_parameter(
        "cfeat", [128, CCH], mybir.dt.float32r, isOutput=False
    )
    rowmax_out = nc.declare_dram_parameter(
        "rowmax", [ROWT, NTILES], mybir.dt.float32, isOutput=True
    )

    with tile.TileContext(nc) as tc:
        with (
            tc.tile_pool(name="feats", bufs=1) as feats,
            tc.tile_pool(name="psum", bufs=2, space="PSUM") as psum_pool,
            tc.tile_pool(name="outp", bufs=1) as outp,
        ):
            # Separate tiles per input chunk so Tile's dependency tracking
            # lets early matmuls start while later chunks are still in
            # flight.  c splits by parity column (r), q by g-quarters.
            c_sbs = [
                feats.tile([128, SPAN], mybir.dt.float32r, tag=f"c{r}", name=f"c{r}")
                for r in range(2)
            ]
            QQ = 4 * ROWT
            q_sbs = [
                feats.tile([128, QQ], mybir.dt.float32r, tag=f"q{i}", name=f"q{i}")
                for i in range(4)
            ]
            nc.sync.dma_start(out=c_sbs[0][:], in_=cfeat_d[:, :SPAN])
            nc.scalar.dma_start(out=q_sbs[0][:], in_=qfeat_d[:, :QQ])
            nc.sync.dma_start(out=q_sbs[1][:], in_=qfeat_d[:, QQ : 2 * QQ])
            nc.scalar.dma_start(out=c_sbs[1][:], in_=cfeat_d[:, SPAN:])
            nc.sync.dma_start(out=q_sbs[2][:], in_=qfeat_d[:, 2 * QQ : 3 * QQ])
            nc.scalar.dma_start(out=q_sbs[3][:], in_=qfeat_d[:, 3 * QQ :])

            rmax = outp.tile([ROWT, NTILES], mybir.dt.float32)

            # 4 row-tiles share one 4-bank PSUM tensor; a single
            # TENSOR_REDUCE with a 3D AP [128, 4, SPAN] reduces all 4
            # (out free size 4), amortizing the per-op PSUM overhead.
            # Even reduce-groups first: they only need c chunk r=0.
            g_order = list(range(0, NGROUP, 2)) + list(range(1, NGROUP, 2))
            for g in g_order:
                r = g % 2
                q_sb = q_sbs[g // 4]
                qcol = ROWT * (g % 4)
                ps = psum_pool.tile([ROWT, 4, 512], mybir.dt.float32, tag="ps", name=f"ps{g}")
                for j in range(4):
                    p0 = 32 * j
                    nc.tensor.matmul(
                        ps[:, j, :SPAN],
                        lhsT=q_sb[p0 : p0 + 32, qcol : qcol + ROWT],
                        rhs=c_sbs[r][p0 : p0 + 32, :],
                        start=True,
                        stop=True,
                        tile_position=(32 * j, 0),
                    )
                nc.vector.reduce_max(
                    rmax[:, 4 * g : 4 * g + 4],
                    ps[:, :, :SPAN],
                    axis=mybir.AxisListType.X,
                )

            nc.sync.dma_start(out=rowmax_out[:], in_=rmax[:])
    nc.compile()
    return nc


def _job_arrays(A, Bset):
    """Build per-row-group gathered feature arrays for one job."""
    ao = np.argsort(A[:, 0], kind="stable")
    bo = np.argsort(Bset[:, 0], kind="stable")
    As = np.ascontiguousarray(A[ao])
    Bs = np.ascontiguousarray(Bset[bo])

    qfeat = np.empty((4, N), np.float32)
    qfeat[0:3] = (2.0 * As).T
    qfeat[3] = 1.0

    cfeat = np.empty((4, PADDED), np.float32)
    cfeat[0:3] = PAD_COORD
    cfeat[3] = -3.0 * PAD_COORD * PAD_COORD
    cfeat[0:3, WL : WL + N] = Bs.T
    cfeat[3, WL : WL + N] = -(Bs.astype(np.float64) ** 2).sum(1).astype(np.float32)

    # c_big interleaved: partition 32j + 4m + f = (feature f, row group j,
    # chunk m), chunk m covering reduce-groups {2m, 2m+1}.
    # q_stat zero-padded stationary: for reduce-group g, row group j, the
    # [32, 128] slice at columns 128g has features only in rows 4*(g//2)..+4.
    q_stat = np.zeros((128, NGROUP * ROWT), np.float32)
    c_big = np.empty((128, 2 * SPAN), np.float32)
    g = np.arange(NGROUP)
    for j in range(4):
        t = 4 * g + j
        cidx = (ROWT * t)[:, None] + np.arange(SPAN)[None, :]
        cj = cfeat[:, cidx]  # [4f, 16g, SPAN]
        c_big[32 * j : 32 * j + 32] = (
            cj.reshape(4, 8, 2 * SPAN).transpose(1, 0, 2).reshape(32, 2 * SPAN)
        )
        for gg in range(NGROUP):
            tt = 4 * gg + j
            m = gg // 2
            q_stat[
                32 * j + 4 * m : 32 * j + 4 * m + 4,
                ROWT * gg : ROWT * gg + ROWT,
            ] = qfeat[:, ROWT * tt : ROWT * tt + ROWT]
    in_map = {"qfeat": q_stat, "cfeat": c_big}
    return As, Bs, in_map


def kernel(pred: np.ndarray, gt: np.ndarray) -> np.ndarray:
    global LAST_RESULTS
    pred = np.asarray(pred, dtype=np.float32)
    gt = np.asarray(gt, dtype=np.float32)
    assert pred.shape == (B, N, 3) and gt.shape == (B, N, 3)

    if "nc" not in _CACHE:
        _CACHE["nc"] = _build_program()
    nc = _CACHE["nc"]

    jobs = []
    in_maps = []
    for b in range(B):
        for A, Bset in ((pred[b], gt[b]), (gt[b], pred[b])):
            As, Bs, in_map = _job_arrays(A, Bset)
            jobs.append((As, Bs))
            in_maps.append(in_map)

    trace = bool(int(os.environ.get("CHAMFER_TRACE", "0")))
    bk = run_bass_kernel_spmd(nc, in_maps, list(range(8)), trace=trace)
    LAST_RESULTS = bk
    results = bk.results

    # Host: undo the rowmax formulation, certify, fix up, and average.
    total = 0.0
    i = np.arange(N)
    t = i // ROWT
    lo = ROWT * t - WL  # window start (unpadded coords, may be < 0)
    hi = ROWT * t + (SPAN - WL)  # window end (may be > N)
    for (As, Bs), r in zip(jobs, results):
        rowmax = np.asarray(r["rowmax"])  # [128, 64]
        asq = (As.astype(np.float64) ** 2).sum(1)
        d_band = asq - rowmax.T.reshape(-1).astype(np.float64)

        bx = Bs[:, 0].astype(np.float64)
        ax = As[:, 0].astype(np.float64)
        lmarg = np.where(lo >= 1, ax - bx[np.clip(lo - 1, 0, N - 1)], np.inf)
        rmarg = np.where(hi < N, bx[np.clip(hi, 0, N - 1)] - ax, np.inf)
        marg = np.minimum(lmarg, rmarg)
        ok = (marg >= 0) & (d_band <= marg * marg)
        bad = np.flatnonzero(~ok)
        if bad.size:
            Ad = As[bad].astype(np.float64)
            Bd = Bs.astype(np.float64)
            d = ((Ad[:, None, :] - Bd[None, :, :]) ** 2).sum(-1)
            d_band[bad] = d.min(1)
        total += d_band.mean()

    return np.float32(total / B)
